# revision 13
# baseline (speedup 1.0000x reference)
"""Trainium2 Bass kernel for causal self-attention with RoPE.

Contract: kernel(**inputs) takes the FULL unsharded inputs of
nn_CausalSelfAttention (x (2,2048,1024) f32, padding_mask (2,2048) bool,
W_kqv (3072,1024), b_kqv (3072,), W_proj (1024,1024), b_proj (1024,))
and returns the full (2,2048,1024) f32 output.

Sharding: 16 heads x 2 batches = 32 (batch, head) pairs, 4 per core
(2 heads, both batches). Each core computes the QKV projection for its
2 heads only (weights pre-sliced host-side), attention for its 4 pairs,
and a partial output projection over its 128 channels of the 1024-wide
contraction. The host sums the 8 partial outputs (no collectives).
"""

import sys

for _p in ("/opt/trn_rl_repo",):
    if _p not in sys.path:
        sys.path.append(_p)

import math

import numpy as np

import concourse.bass as bass
import concourse.tile as tile
from concourse import bacc, mybir
from concourse.bass_utils import run_bass_kernel_spmd

# Problem constants (hardcoded per spec).
B, T, C = 2, 2048, 1024
H, D = 16, 64
N_CORES = 8
HPC = H // N_CORES          # heads per core = 2
TT = B * T                  # 4096
NCT = C // 128              # 8 c-tiles
CHUNK = 512                 # t-chunk width (phase A) and q-chunk width (phase B)
NCHUNK = TT // CHUNK        # 8
KT_PER_B = T // 128         # 16 k-tiles per batch
SCALE = 1.0 / math.sqrt(D)
ROPE_BASE = 10000.0

F32 = mybir.dt.float32
MMDT = mybir.dt.float32r    # reduced-precision fp32 matmul mode (4x faster)

_CACHE = {}


def _rope_tables():
    half = D // 2
    inv_freq = (np.float32(ROPE_BASE) ** (-(np.arange(half, dtype=np.float32) / np.float32(half)))).astype(np.float32)
    t = np.arange(T, dtype=np.float32)[:, None]
    ang = t * inv_freq[None, :]                       # (T, 32)
    sin = np.concatenate([np.sin(ang), np.sin(ang)], axis=-1)  # (T, 64)
    cos = np.concatenate([np.cos(ang), np.cos(ang)], axis=-1)
    # transpose to (64, T), duplicate along partitions for the 2 heads of an M-tile
    sinT = np.ascontiguousarray(np.concatenate([sin.T, sin.T], axis=0))  # (128, T)
    cosT = np.ascontiguousarray(np.concatenate([cos.T, cos.T], axis=0))
    return sinT, cosT


def _numpy_fallback(x, padding_mask, W_kqv, b_kqv, W_proj, b_proj):
    """Exact reference in numpy — used only for non-all-ones padding masks."""
    x = np.asarray(x, np.float32)
    qkv = x.reshape(TT, C) @ np.asarray(W_kqv, np.float32).T + np.asarray(b_kqv, np.float32)
    qkv = qkv.reshape(B, T, 3 * C)
    k, q, v = np.split(qkv, 3, axis=-1)

    def split_heads(t):
        return t.reshape(B, T, H, D).transpose(0, 2, 1, 3)

    q, k, v = split_heads(q), split_heads(k), split_heads(v)
    sinT, cosT = _rope_tables()
    sin = sinT[:D].T[None, None]
    cos = cosT[:D].T[None, None]

    def rot(t):
        return np.concatenate([-t[..., D // 2:], t[..., : D // 2]], axis=-1)

    q = q * cos + rot(q) * sin
    k = k * cos + rot(k) * sin
    att = np.einsum("bhqd,bhkd->bhqk", q, k) * SCALE
    causal = np.tril(np.ones((T, T), bool))[None, None]
    mask = causal & np.asarray(padding_mask)[:, None, None, :]
    att = np.where(mask, att, -np.inf)
    att = att - att.max(axis=-1, keepdims=True)
    e = np.exp(att)
    p = e / e.sum(axis=-1, keepdims=True)
    y = np.einsum("bhqk,bhkd->bhqd", p, v)
    y = y * np.asarray(padding_mask)[:, None, :, None]
    y = y.transpose(0, 2, 1, 3).reshape(B, T, C)
    return (y @ np.asarray(W_proj, np.float32).T + np.asarray(b_proj, np.float32)).astype(np.float32)


def build_program():
    nc = bacc.Bacc("TRN2", target_bir_lowering=False, debug=False, num_devices=N_CORES)

    # ---- I/O ----
    xT = nc.dram_tensor("xT", [C, TT], MMDT, kind="ExternalInput").ap()
    w_sb_d = nc.dram_tensor("w_sb", [128, NCT * 384], MMDT, kind="ExternalInput").ap()
    b_sb_d = nc.dram_tensor("b_sb", [128, 3], F32, kind="ExternalInput").ap()
    wproj_d = nc.dram_tensor("wprojT", [128, C], MMDT, kind="ExternalInput").ap()
    cos_d = nc.dram_tensor("cosT", [128, T], MMDT, kind="ExternalInput").ap()
    sin_d = nc.dram_tensor("sinT", [128, T], MMDT, kind="ExternalInput").ap()
    rperm_d = nc.dram_tensor("rperm", [128, 128], MMDT, kind="ExternalInput").ap()
    dmask_d = nc.dram_tensor("dmask", [128, 4 * 512], MMDT, kind="ExternalInput").ap()
    ident_d = nc.dram_tensor("ident", [128, 128], MMDT, kind="ExternalInput").ap()
    identd_d = nc.dram_tensor("ident_dup", [128, 64], MMDT, kind="ExternalInput").ap()
    ones_d = nc.dram_tensor("ones_col", [128, 4 * KT_PER_B, 32], MMDT, kind="ExternalInput").ap()
    out_d = nc.dram_tensor("out_part", [TT, C], F32, kind="ExternalOutput").ap()

    Exp = mybir.ActivationFunctionType.Exp
    Copy = mybir.ActivationFunctionType.Copy

    with tile.TileContext(nc) as tc:
        with (
            tc.tile_pool(name="const", bufs=1) as cpool,
            tc.tile_pool(name="persist", bufs=1) as ppool,
            tc.tile_pool(name="outT", bufs=2) as opool,
            tc.tile_pool(name="work", bufs=3) as wpool,
            tc.tile_pool(name="ptp", bufs=4) as ptp,
            tc.tile_pool(name="xtp", bufs=12) as xtp,
            tc.tile_pool(name="work2", bufs=2) as w2pool,
            tc.tile_pool(name="psum", bufs=2, space="PSUM") as ps,
        ):
            # ---- weights first (matmuls start as soon as block 0 lands) ----
            w_t = []
            for ct in range(NCT):
                wt = cpool.tile([128, 384], MMDT, tag=f"w{ct}")
                nc.sync.dma_start(wt[:], w_sb_d[:, ct * 384:(ct + 1) * 384])
                w_t.append(wt)
            b_sb = cpool.tile([128, 3], F32, tag="b")
            nc.sync.dma_start(b_sb[:], b_sb_d[:])
            cosT = cpool.tile([128, T], MMDT, tag="cos")
            nc.sync.dma_start(cosT[:], cos_d[:])
            sinT = cpool.tile([128, T], MMDT, tag="sin")
            nc.sync.dma_start(sinT[:], sin_d[:])
            rperm = cpool.tile([128, 128], MMDT, tag="rp")
            nc.sync.dma_start(rperm[:], rperm_d[:])
            identd = cpool.tile([128, 64], MMDT, tag="idd")
            nc.sync.dma_start(identd[:], identd_d[:])
            wproj = cpool.tile([128, C], MMDT, tag="wp")
            nc.sync.dma_start(wproj[:], wproj_d[:])
            dmask = cpool.tile([128, 4 * 512], MMDT, tag="dm")
            nc.sync.dma_start(dmask[:], dmask_d[:])
            ident = cpool.tile([128, 128], MMDT, tag="id")
            nc.sync.dma_start(ident[:], ident_d[:])

            # ---- persistent buffers ----
            kT_buf = ppool.tile([128, TT], MMDT, tag="kT")
            qT_buf = ppool.tile([128, TT], MMDT, tag="qT")
            v_ext = ppool.tile([128, 4 * KT_PER_B, 96], MMDT, tag="vx")
            yT_buf = ppool.tile([128, TT], MMDT, tag="yT")

            # ones column for the softmax-denominator trick
            nc.sync.dma_start(v_ext[:, :, 64:96], ones_d[:])

            # ================= Phase A: projection + RoPE + V layout ==========
            for i in range(NCHUNK):
                b = i // (NCHUNK // B)
                tb = (i % (NCHUNK // B)) * CHUNK  # within-batch t offset
                ps_kq = ps.tile([128, 1024], F32, tag="big")
                ps_v = ps.tile([128, 512], F32, tag="tr")
                for ct in range(NCT):
                    xt = xtp.tile([128, CHUNK], MMDT, tag="xt")
                    nc.sync.dma_start(xt[:], xT[ct * 128:(ct + 1) * 128, i * CHUNK:(i + 1) * CHUNK])
                    st, sp = (ct == 0), (ct == NCT - 1)
                    nc.tensor.matmul(ps_kq[:, 0:512], w_t[ct][:, 0:128], xt[:], start=st, stop=sp)
                    nc.tensor.matmul(ps_kq[:, 512:1024], w_t[ct][:, 128:256], xt[:], start=st, stop=sp)
                    nc.tensor.matmul(ps_v[:], w_t[ct][:, 256:384], xt[:], start=st, stop=sp)

                k_raw = w2pool.tile([128, CHUNK], MMDT, tag="kraw")
                q_raw = w2pool.tile([128, CHUNK], MMDT, tag="qraw")
                v_raw = w2pool.tile([128, CHUNK], MMDT, tag="vraw")
                nc.vector.tensor_scalar_add(k_raw[:], ps_kq[:, 0:512], b_sb[:, 0:1])
                nc.vector.tensor_scalar_add(q_raw[:], ps_kq[:, 512:1024], b_sb[:, 1:2])
                nc.vector.tensor_scalar_add(v_raw[:], ps_v[:], b_sb[:, 2:3])

                # rotate_half via +-1 permutation matmul, then q' = q*cos + rot*sin
                ps_krot = ps.tile([128, CHUNK], F32, tag="av")
                ps_qrot = ps.tile([128, CHUNK], F32, tag="av")
                nc.tensor.matmul(ps_krot[:], rperm[:], k_raw[:], start=True, stop=True)
                nc.tensor.matmul(ps_qrot[:], rperm[:], q_raw[:], start=True, stop=True)
                cs = cosT[:, tb:tb + CHUNK]
                sn = sinT[:, tb:tb + CHUNK]
                tmp1 = w2pool.tile([128, CHUNK], MMDT, tag="tmp1")
                tmp2 = w2pool.tile([128, CHUNK], MMDT, tag="tmp2")
                nc.vector.tensor_mul(tmp1[:], k_raw[:], cs)
                nc.vector.tensor_mul(tmp2[:], ps_krot[:], sn)
                nc.vector.tensor_add(kT_buf[:, i * CHUNK:(i + 1) * CHUNK], tmp1[:], tmp2[:])
                tmp3 = w2pool.tile([128, CHUNK], MMDT, tag="tmp1")
                tmp4 = w2pool.tile([128, CHUNK], MMDT, tag="tmp2")
                nc.vector.tensor_mul(tmp3[:], q_raw[:], cs)
                nc.vector.tensor_mul(tmp4[:], ps_qrot[:], sn)
                nc.vector.tensor_add(qT_buf[:, i * CHUNK:(i + 1) * CHUNK], tmp3[:], tmp4[:])

                # V: transpose (64,128) slabs into v_ext natural layout (batched copy)
                for hl in range(HPC):
                    p = b * HPC + hl
                    kt0 = (i % (NCHUNK // B)) * (CHUNK // 128)  # first k-tile of chunk
                    vtr = ps.tile([128, 4, 64], MMDT, tag="tr")
                    for j in range(CHUNK // 128):
                        nc.tensor.transpose(
                            vtr[:, j, :], v_raw[hl * 64:(hl + 1) * 64, j * 128:(j + 1) * 128],
                            identd[hl * 64:(hl + 1) * 64, :],
                        )
                    nc.vector.tensor_copy(
                        v_ext[:, p * KT_PER_B + kt0:p * KT_PER_B + kt0 + 4, 0:64], vtr[:])

            # ================= Phase B/C: attention + output projection =======
            def emit_outproj(tg):
                out_sb = wpool.tile([128, C], F32, tag="osb")
                for half in range(2):
                    op_ps = ps.tile([128, 512], F32, tag="tr")
                    nc.tensor.matmul(
                        op_ps[:],
                        yT_buf[:, tg * 128:(tg + 1) * 128],
                        wproj[:, half * 512:(half + 1) * 512],
                        start=True, stop=True,
                    )
                    if half == 0:
                        nc.vector.tensor_copy(out_sb[:, 0:512], op_ps[:])
                    else:
                        nc.scalar.activation(out_sb[:, 512:1024], op_ps[:], Copy)
                nc.sync.dma_start(out_d[tg * 128:(tg + 1) * 128, :], out_sb[:])

            for b in range(B):
                for hl in range(HPC):
                    p = b * HPC + hl
                    QT = qT_buf[hl * 64:(hl + 1) * 64, b * T:(b + 1) * T]
                    KT = kT_buf[hl * 64:(hl + 1) * 64, b * T:(b + 1) * T]
                    outT = opool.tile([96, T], MMDT, tag="outT")
                    for qc in range(T // CHUNK):
                        n_kt = 4 * (qc + 1)
                        av_ps = ps.tile([96, CHUNK], F32, tag="av")
                        # software-pipelined: S/exp/mask of group g, AV lags 2 groups
                        pending = []  # [(pt, kjs), ...]

                        def flush_av(drain):
                            while pending and (len(pending) > 2 or drain):
                                ppt, pkjs = pending.pop(0)
                                for idx, kj in enumerate(pkjs):
                                    nc.tensor.matmul(
                                        av_ps[:],
                                        v_ext[:, p * KT_PER_B + kj, :],
                                        ppt[:, idx * 512:(idx + 1) * 512],
                                        start=(kj == 0), stop=(kj == n_kt - 1),
                                    )

                        for g0 in range(0, n_kt, 2):
                            kjs = list(range(g0, min(g0 + 2, n_kt)))
                            gw = len(kjs) * 512
                            s_ps = ps.tile([128, 1024], F32, tag="big")
                            for idx, kj in enumerate(kjs):
                                nc.tensor.matmul(
                                    s_ps[:, idx * 512:(idx + 1) * 512],
                                    KT[:, kj * 128:(kj + 1) * 128],
                                    QT[:, qc * CHUNK:(qc + 1) * CHUNK],
                                    start=True, stop=True,
                                )
                            pt = ptp.tile([128, 1024], MMDT, tag="pt")
                            nc.scalar.activation(pt[:, 0:gw], s_ps[:, 0:gw], Exp, scale=SCALE)
                            for idx, kj in enumerate(kjs):
                                if kj >= 4 * qc:  # diagonal-region k-tile
                                    dcol = kj - 4 * qc
                                    off = idx * 512
                                    w = (dcol + 1) * 128
                                    nc.gpsimd.tensor_mul(
                                        pt[:, off:off + w],
                                        pt[:, off:off + w],
                                        dmask[:, dcol * 512:dcol * 512 + w],
                                    )
                            pending.append((pt, kjs))
                            flush_av(False)
                        flush_av(True)
                        nc.vector.tensor_copy(outT[:, qc * CHUNK:(qc + 1) * CHUNK], av_ps[:])
                        # bury exp latency of batch-1 attention under batch-0 out-proj
                        if b == 1:
                            slot = hl * 4 + qc
                            for tg in (2 * slot, 2 * slot + 1):
                                emit_outproj(tg)

                    # pair tail: transpose, normalize, transpose back into yT_buf
                    for jg in range(T // 512):
                        nat = ps.tile([128, 4, 96], MMDT, tag="tr")
                        for j4 in range(4):
                            nc.tensor.transpose(
                                nat[:, j4, :], outT[:, (jg * 4 + j4) * 128:(jg * 4 + j4 + 1) * 128],
                                ident[0:96, 0:96])
                        recip = wpool.tile([128, 4, 1], F32, tag="rcp")
                        nc.vector.reciprocal(recip[:], nat[:, :, 64:65])
                        y_nat = wpool.tile([128, 4, 64], MMDT, tag="ynat")
                        for j4 in range(4):
                            nc.vector.tensor_scalar_mul(
                                y_nat[:, j4, :], nat[:, j4, 0:64], recip[:, j4, :])
                        yt_ps = ps.tile([64, 512], MMDT, tag="tr")
                        for j4 in range(4):
                            nc.tensor.transpose(yt_ps[:, j4 * 128:(j4 + 1) * 128], y_nat[:, j4, :], ident[:])
                        nc.vector.tensor_copy(
                            yT_buf[hl * 64:(hl + 1) * 64, b * T + jg * 512:b * T + (jg + 1) * 512],
                            yt_ps[:],
                        )

                # ---- batch-1 output projection at the very end ----
                if b == 1:
                    for ti in range(T // 128):
                        emit_outproj(T // 128 + ti)

    nc.compile()
    return nc


def _host_inputs(x, W_kqv, b_kqv, W_proj):
    """Build per-core input maps (host-side slicing/transposition)."""
    x = np.asarray(x, np.float32)
    W_kqv = np.asarray(W_kqv, np.float32)
    b_kqv = np.asarray(b_kqv, np.float32)
    W_proj = np.asarray(W_proj, np.float32)

    xT = np.ascontiguousarray(x.reshape(TT, C).T)  # (1024, 4096)
    sinT, cosT = _rope_tables()

    rmat = np.zeros((128, 128), np.float32)
    for hb in range(2):
        base = hb * 64
        for l in range(32):
            rmat[base + l, base + 32 + l] = -1.0
            rmat[base + 32 + l, base + l] = 1.0
    rperm = np.ascontiguousarray(rmat.T)  # lhsT

    # dmask[dcol]: (128, 512); cols [0, dcol*128) zero, diag block triu, rest ones
    dmask = np.zeros((128, 4 * 512), np.float32)
    triu = np.triu(np.ones((128, 128), np.float32))
    for dcol in range(4):
        base = dcol * 512
        dmask[:, base + dcol * 128: base + (dcol + 1) * 128] = triu
        dmask[:, base + (dcol + 1) * 128: base + 512] = 1.0
    ident = np.eye(128, dtype=np.float32)
    ident_dup = np.ascontiguousarray(
        np.concatenate([np.eye(64, dtype=np.float32), np.eye(64, dtype=np.float32)], axis=0))
    ones_col = np.zeros((128, 4 * KT_PER_B, 32), np.float32)
    ones_col[:, :, 0] = 1.0

    shared = {
        "xT": xT, "cosT": cosT, "sinT": sinT, "rperm": rperm,
        "dmask": dmask, "ident": ident, "ident_dup": ident_dup,
        "ones_col": ones_col,
    }

    in_maps = []
    for c in range(N_CORES):
        heads = [HPC * c + h for h in range(HPC)]

        def rows(mat, base):
            return np.concatenate([mat[base + h * D: base + (h + 1) * D] for h in heads], axis=0)

        Wk, Wq, Wv = rows(W_kqv, 0), rows(W_kqv, C), rows(W_kqv, 2 * C)
        w_sb = np.empty((128, NCT * 384), np.float32)
        for ct in range(NCT):
            sl = slice(ct * 128, (ct + 1) * 128)
            w_sb[:, ct * 384 + 0:ct * 384 + 128] = Wk[:, sl].T
            w_sb[:, ct * 384 + 128:ct * 384 + 256] = Wq[:, sl].T
            w_sb[:, ct * 384 + 256:ct * 384 + 384] = Wv[:, sl].T
        b_core = np.stack([
            np.concatenate([b_kqv[0 * C + h * D: 0 * C + (h + 1) * D] for h in heads]),
            np.concatenate([b_kqv[1 * C + h * D: 1 * C + (h + 1) * D] for h in heads]),
            np.concatenate([b_kqv[2 * C + h * D: 2 * C + (h + 1) * D] for h in heads]),
        ], axis=1).astype(np.float32)  # (128, 3)
        ch = np.concatenate([np.arange(h * D, (h + 1) * D) for h in heads])
        wprojT = np.ascontiguousarray(W_proj[:, ch].T)  # (128, 1024)

        in_maps.append({**shared, "w_sb": np.ascontiguousarray(w_sb),
                        "b_sb": b_core, "wprojT": wprojT})
    return in_maps


def kernel(x, padding_mask, W_kqv, b_kqv, W_proj, b_proj):
    padding_mask = np.asarray(padding_mask)
    if not padding_mask.all():
        return _numpy_fallback(x, padding_mask, W_kqv, b_kqv, W_proj, b_proj)

    if "nc" not in _CACHE:
        _CACHE["nc"] = build_program()
    nc = _CACHE["nc"]

    in_maps = _host_inputs(x, W_kqv, b_kqv, W_proj)
    res = run_bass_kernel_spmd(nc, in_maps, core_ids=list(range(N_CORES)))
    out = np.zeros((TT, C), np.float64)
    for c in range(N_CORES):
        out += res.results[c]["out_part"]
    out = (out + np.asarray(b_proj, np.float64)).astype(np.float32)
    return out.reshape(B, T, C)


if __name__ == "__main__":
    rng = np.random.default_rng(0)
    x = rng.standard_normal((B, T, C), dtype=np.float32)
    pm = np.ones((B, T), bool)
    W_kqv = (rng.standard_normal((3 * C, C), dtype=np.float32) / math.sqrt(C)).astype(np.float32)
    b_kqv = np.zeros((3 * C,), np.float32)
    W_proj = (rng.standard_normal((C, C), dtype=np.float32) / math.sqrt(C)).astype(np.float32)
    b_proj = np.zeros((C,), np.float32)
    out = kernel(x, pm, W_kqv, b_kqv, W_proj, b_proj)
    ref = _numpy_fallback(x, pm, W_kqv, b_kqv, W_proj, b_proj)
    err = np.abs(out - ref).max() / np.abs(ref).max()
    print("self-check rel err:", err)


# revision 18
# speedup vs baseline: 1.1982x; 1.1982x over previous
"""Trainium2 Bass kernel for causal self-attention with RoPE.

Contract: kernel(**inputs) takes the FULL unsharded inputs of
nn_CausalSelfAttention (x (2,2048,1024) f32, padding_mask (2,2048) bool,
W_kqv (3072,1024), b_kqv (3072,), W_proj (1024,1024), b_proj (1024,))
and returns the full (2,2048,1024) f32 output.

Sharding: 16 heads x 2 batches = 32 (batch, head) pairs, 4 per core
(2 heads, both batches). Each core computes the QKV projection for its
2 heads only (weights pre-sliced host-side), attention for its 4 pairs,
and a partial output projection over its 128 channels of the 1024-wide
contraction. The host sums the 8 partial outputs (no collectives).
"""

import sys

for _p in ("/opt/trn_rl_repo",):
    if _p not in sys.path:
        sys.path.append(_p)

import math

import numpy as np

import concourse.bass as bass
import concourse.tile as tile
from concourse import bacc, mybir
from concourse.bass_utils import run_bass_kernel_spmd

# Problem constants (hardcoded per spec).
B, T, C = 2, 2048, 1024
H, D = 16, 64
N_CORES = 8
HPC = H // N_CORES          # heads per core = 2
TT = B * T                  # 4096
NCT = C // 128              # 8 c-tiles
CHUNK = 512                 # t-chunk width (phase A) and q-chunk width (phase B)
NCHUNK = TT // CHUNK        # 8
KT_PER_B = T // 128         # 16 k-tiles per batch
SCALE = 1.0 / math.sqrt(D)
ROPE_BASE = 10000.0

F32 = mybir.dt.float32
MMDT = mybir.dt.float32r    # reduced-precision fp32 matmul mode (4x faster)

_CACHE = {}


def _rope_tables():
    half = D // 2
    inv_freq = (np.float32(ROPE_BASE) ** (-(np.arange(half, dtype=np.float32) / np.float32(half)))).astype(np.float32)
    t = np.arange(T, dtype=np.float32)[:, None]
    ang = t * inv_freq[None, :]                       # (T, 32)
    sin = np.concatenate([np.sin(ang), np.sin(ang)], axis=-1)  # (T, 64)
    cos = np.concatenate([np.cos(ang), np.cos(ang)], axis=-1)
    # transpose to (64, T), duplicate along partitions for the 2 heads of an M-tile
    sinT = np.ascontiguousarray(np.concatenate([sin.T, sin.T], axis=0))  # (128, T)
    cosT = np.ascontiguousarray(np.concatenate([cos.T, cos.T], axis=0))
    return sinT, cosT


def _numpy_fallback(x, padding_mask, W_kqv, b_kqv, W_proj, b_proj):
    """Exact reference in numpy — used only for non-all-ones padding masks."""
    x = np.asarray(x, np.float32)
    qkv = x.reshape(TT, C) @ np.asarray(W_kqv, np.float32).T + np.asarray(b_kqv, np.float32)
    qkv = qkv.reshape(B, T, 3 * C)
    k, q, v = np.split(qkv, 3, axis=-1)

    def split_heads(t):
        return t.reshape(B, T, H, D).transpose(0, 2, 1, 3)

    q, k, v = split_heads(q), split_heads(k), split_heads(v)
    sinT, cosT = _rope_tables()
    sin = sinT[:D].T[None, None]
    cos = cosT[:D].T[None, None]

    def rot(t):
        return np.concatenate([-t[..., D // 2:], t[..., : D // 2]], axis=-1)

    q = q * cos + rot(q) * sin
    k = k * cos + rot(k) * sin
    att = np.einsum("bhqd,bhkd->bhqk", q, k) * SCALE
    causal = np.tril(np.ones((T, T), bool))[None, None]
    mask = causal & np.asarray(padding_mask)[:, None, None, :]
    att = np.where(mask, att, -np.inf)
    att = att - att.max(axis=-1, keepdims=True)
    e = np.exp(att)
    p = e / e.sum(axis=-1, keepdims=True)
    y = np.einsum("bhqk,bhkd->bhqd", p, v)
    y = y * np.asarray(padding_mask)[:, None, :, None]
    y = y.transpose(0, 2, 1, 3).reshape(B, T, C)
    return (y @ np.asarray(W_proj, np.float32).T + np.asarray(b_proj, np.float32)).astype(np.float32)


def build_program():
    nc = bacc.Bacc("TRN2", target_bir_lowering=False, debug=False, num_devices=N_CORES)

    # ---- I/O ----
    xT = nc.dram_tensor("xT", [C, TT], MMDT, kind="ExternalInput").ap()
    w_sb_d = nc.dram_tensor("w_sb", [128, NCT * 384], MMDT, kind="ExternalInput").ap()
    b_sb_d = nc.dram_tensor("b_sb", [128, 3], F32, kind="ExternalInput").ap()
    wproj_d = nc.dram_tensor("wprojT", [128, C], MMDT, kind="ExternalInput").ap()
    cos_d = nc.dram_tensor("cosT", [128, T], MMDT, kind="ExternalInput").ap()
    sin_d = nc.dram_tensor("sinT", [128, T], MMDT, kind="ExternalInput").ap()
    rperm_d = nc.dram_tensor("rperm", [128, 128], MMDT, kind="ExternalInput").ap()
    dmask_d = nc.dram_tensor("dmask", [128, 4 * 512], MMDT, kind="ExternalInput").ap()
    ident_d = nc.dram_tensor("ident", [128, 128], MMDT, kind="ExternalInput").ap()
    identd_d = nc.dram_tensor("ident_dup", [128, 64], MMDT, kind="ExternalInput").ap()
    ones_d = nc.dram_tensor("ones_col", [128, 4 * KT_PER_B, 32], MMDT, kind="ExternalInput").ap()
    out_d = nc.dram_tensor("out_part", [TT, C], F32, kind="ExternalOutput").ap()

    Exp = mybir.ActivationFunctionType.Exp
    Copy = mybir.ActivationFunctionType.Copy

    with tile.TileContext(nc) as tc:
        with (
            tc.tile_pool(name="const", bufs=1) as cpool,
            tc.tile_pool(name="persist", bufs=1) as ppool,
            tc.tile_pool(name="outT", bufs=2) as opool,
            tc.tile_pool(name="work", bufs=3) as wpool,
            tc.tile_pool(name="ptp", bufs=4) as ptp,
            tc.tile_pool(name="xtp", bufs=12) as xtp,
            tc.tile_pool(name="work2", bufs=2) as w2pool,
            tc.tile_pool(name="psum", bufs=2, space="PSUM") as ps,
            tc.tile_pool(name="psum_av", bufs=1, space="PSUM") as ps_av,
        ):
            # ---- weights first (matmuls start as soon as block 0 lands) ----
            w_t = []
            for ct in range(NCT):
                wt = cpool.tile([128, 384], MMDT, tag=f"w{ct}")
                nc.sync.dma_start(wt[:], w_sb_d[:, ct * 384:(ct + 1) * 384])
                w_t.append(wt)
            b_sb = cpool.tile([128, 3], F32, tag="b")
            nc.sync.dma_start(b_sb[:], b_sb_d[:])
            cosT = cpool.tile([128, T], MMDT, tag="cos")
            nc.sync.dma_start(cosT[:], cos_d[:])
            sinT = cpool.tile([128, T], MMDT, tag="sin")
            nc.sync.dma_start(sinT[:], sin_d[:])
            rperm = cpool.tile([128, 128], MMDT, tag="rp")
            nc.sync.dma_start(rperm[:], rperm_d[:])
            identd = cpool.tile([128, 64], MMDT, tag="idd")
            nc.sync.dma_start(identd[:], identd_d[:])
            wproj = cpool.tile([128, C], MMDT, tag="wp")
            nc.sync.dma_start(wproj[:], wproj_d[:])
            dmask = cpool.tile([128, 4 * 512], MMDT, tag="dm")
            nc.sync.dma_start(dmask[:], dmask_d[:])
            ident = cpool.tile([128, 128], MMDT, tag="id")
            nc.sync.dma_start(ident[:], ident_d[:])

            # ---- persistent buffers ----
            kT_buf = ppool.tile([128, TT], MMDT, tag="kT")
            qT_buf = ppool.tile([128, TT], MMDT, tag="qT")
            v_ext = ppool.tile([128, 4 * KT_PER_B, 96], MMDT, tag="vx")
            yT_buf = ppool.tile([128, TT], MMDT, tag="yT")

            # ones column for the softmax-denominator trick
            nc.sync.dma_start(v_ext[:, :, 64:96], ones_d[:])

            # ================= Phase A: projection + RoPE + V layout ==========
            for i in range(NCHUNK):
                b = i // (NCHUNK // B)
                tb = (i % (NCHUNK // B)) * CHUNK  # within-batch t offset
                ps_kq = ps.tile([128, 1024], F32, tag="big")
                ps_v = ps.tile([128, 512], F32, tag="tr")
                for ct in range(NCT):
                    xt = xtp.tile([128, CHUNK], MMDT, tag="xt")
                    nc.sync.dma_start(xt[:], xT[ct * 128:(ct + 1) * 128, i * CHUNK:(i + 1) * CHUNK])
                    st, sp = (ct == 0), (ct == NCT - 1)
                    nc.tensor.matmul(ps_kq[:, 0:512], w_t[ct][:, 0:128], xt[:], start=st, stop=sp)
                    nc.tensor.matmul(ps_kq[:, 512:1024], w_t[ct][:, 128:256], xt[:], start=st, stop=sp)
                    nc.tensor.matmul(ps_v[:], w_t[ct][:, 256:384], xt[:], start=st, stop=sp)

                k_raw = w2pool.tile([128, CHUNK], MMDT, tag="kraw")
                q_raw = w2pool.tile([128, CHUNK], MMDT, tag="qraw")
                v_raw = w2pool.tile([128, CHUNK], MMDT, tag="vraw")
                nc.vector.tensor_scalar_add(k_raw[:], ps_kq[:, 0:512], b_sb[:, 0:1])
                nc.vector.tensor_scalar_add(q_raw[:], ps_kq[:, 512:1024], b_sb[:, 1:2])
                nc.vector.tensor_scalar_add(v_raw[:], ps_v[:], b_sb[:, 2:3])

                # rotate_half via +-1 permutation matmul, then q' = q*cos + rot*sin
                ps_rot = ps_av.tile([128, 1024], F32, tag="av")
                ps_krot = ps_rot[:, 0:512]
                ps_qrot = ps_rot[:, 512:1024]
                nc.tensor.matmul(ps_krot[:], rperm[:], k_raw[:], start=True, stop=True)
                nc.tensor.matmul(ps_qrot[:], rperm[:], q_raw[:], start=True, stop=True)
                cs = cosT[:, tb:tb + CHUNK]
                sn = sinT[:, tb:tb + CHUNK]
                tmp1 = w2pool.tile([128, CHUNK], MMDT, tag="tmp1")
                tmp2 = w2pool.tile([128, CHUNK], MMDT, tag="tmp2")
                nc.vector.tensor_mul(tmp1[:], k_raw[:], cs)
                nc.vector.tensor_mul(tmp2[:], ps_krot[:], sn)
                nc.vector.tensor_add(kT_buf[:, i * CHUNK:(i + 1) * CHUNK], tmp1[:], tmp2[:])
                tmp3 = w2pool.tile([128, CHUNK], MMDT, tag="tmp1")
                tmp4 = w2pool.tile([128, CHUNK], MMDT, tag="tmp2")
                nc.vector.tensor_mul(tmp3[:], q_raw[:], cs)
                nc.vector.tensor_mul(tmp4[:], ps_qrot[:], sn)
                nc.vector.tensor_add(qT_buf[:, i * CHUNK:(i + 1) * CHUNK], tmp3[:], tmp4[:])

                # V: transpose (64,128) slabs into v_ext natural layout (batched copy)
                for hl in range(HPC):
                    p = b * HPC + hl
                    kt0 = (i % (NCHUNK // B)) * (CHUNK // 128)  # first k-tile of chunk
                    vtr = ps.tile([128, 4, 64], MMDT, tag="tr")
                    for j in range(CHUNK // 128):
                        nc.tensor.transpose(
                            vtr[:, j, :], v_raw[hl * 64:(hl + 1) * 64, j * 128:(j + 1) * 128],
                            identd[hl * 64:(hl + 1) * 64, :],
                        )
                    nc.vector.tensor_copy(
                        v_ext[:, p * KT_PER_B + kt0:p * KT_PER_B + kt0 + 4, 0:64], vtr[:])

            # ================= Phase B/C: attention + output projection =======
            def emit_outproj(tg):
                out_sb = wpool.tile([128, C], F32, tag="osb")
                for half in range(2):
                    op_ps = ps.tile([128, 512], F32, tag="tr")
                    nc.tensor.matmul(
                        op_ps[:],
                        yT_buf[:, tg * 128:(tg + 1) * 128],
                        wproj[:, half * 512:(half + 1) * 512],
                        start=True, stop=True,
                    )
                    if half == 0:
                        nc.vector.tensor_copy(out_sb[:, 0:512], op_ps[:])
                    else:
                        nc.scalar.activation(out_sb[:, 512:1024], op_ps[:], Copy)
                nc.sync.dma_start(out_d[tg * 128:(tg + 1) * 128, :], out_sb[:])

            for b in range(B):
                for hl in range(HPC):
                    p = b * HPC + hl
                    QT = qT_buf[hl * 64:(hl + 1) * 64, b * T:(b + 1) * T]
                    KT = kT_buf[hl * 64:(hl + 1) * 64, b * T:(b + 1) * T]
                    outT = opool.tile([96, T], MMDT, tag="outT")
                    for qc in range(T // CHUNK):
                        n_kt = 4 * (qc + 1)
                        # double-banked accumulator: even k-tiles -> bank A, odd -> bank B
                        av_ps = ps_av.tile([96, 1024], F32, tag="av")
                        # software-pipelined: S/exp/mask of group g, AV lags 1 group
                        pending = []  # [(pt, kjs), ...]

                        def flush_av(drain):
                            while pending and (len(pending) > 1 or drain):
                                ppt, pkjs = pending.pop(0)
                                for idx, kj in enumerate(pkjs):
                                    bank = (kj % 2) * 512
                                    nc.tensor.matmul(
                                        av_ps[:, bank:bank + 512],
                                        v_ext[:, p * KT_PER_B + kj, :],
                                        ppt[:, idx * 512:(idx + 1) * 512],
                                        start=(kj < 2), stop=(kj >= n_kt - 2),
                                        skip_group_check=True,
                                    )

                        for g0 in range(0, n_kt, 2):
                            kjs = list(range(g0, min(g0 + 2, n_kt)))
                            gw = len(kjs) * 512
                            s_ps = ps.tile([128, 1024], F32, tag="big")
                            for idx, kj in enumerate(kjs):
                                nc.tensor.matmul(
                                    s_ps[:, idx * 512:(idx + 1) * 512],
                                    KT[:, kj * 128:(kj + 1) * 128],
                                    QT[:, qc * CHUNK:(qc + 1) * CHUNK],
                                    start=True, stop=True,
                                )
                            pt = ptp.tile([128, 1024], MMDT, tag="pt")
                            nc.scalar.activation(pt[:, 0:gw], s_ps[:, 0:gw], Exp, scale=SCALE)
                            for idx, kj in enumerate(kjs):
                                if kj >= 4 * qc:  # diagonal-region k-tile
                                    dcol = kj - 4 * qc
                                    off = idx * 512
                                    w = (dcol + 1) * 128
                                    nc.gpsimd.tensor_mul(
                                        pt[:, off:off + w],
                                        pt[:, off:off + w],
                                        dmask[:, dcol * 512:dcol * 512 + w],
                                    )
                            pending.append((pt, kjs))
                            flush_av(False)
                        flush_av(True)
                        oslice = outT[:, qc * CHUNK:(qc + 1) * CHUNK]
                        nc.vector.tensor_copy(oslice, av_ps[:, 0:512])
                        nc.vector.tensor_add(oslice, oslice, av_ps[:, 512:1024])
                        # bury exp latency of batch-1 attention under batch-0 out-proj
                        if b == 1:
                            slot = hl * 4 + qc
                            for tg in (2 * slot, 2 * slot + 1):
                                emit_outproj(tg)

                    # pair tail: transpose, normalize, transpose back into yT_buf
                    for jg in range(T // 512):
                        nat = ps.tile([128, 4, 96], MMDT, tag="tr")
                        for j4 in range(4):
                            nc.tensor.transpose(
                                nat[:, j4, :], outT[:, (jg * 4 + j4) * 128:(jg * 4 + j4 + 1) * 128],
                                ident[0:96, 0:96])
                        recip = wpool.tile([128, 4, 1], F32, tag="rcp")
                        nc.vector.reciprocal(recip[:], nat[:, :, 64:65])
                        y_nat = wpool.tile([128, 4, 64], MMDT, tag="ynat")
                        for j4 in range(4):
                            nc.vector.tensor_scalar_mul(
                                y_nat[:, j4, :], nat[:, j4, 0:64], recip[:, j4, :])
                        yt_ps = ps.tile([64, 512], MMDT, tag="tr")
                        for j4 in range(4):
                            nc.tensor.transpose(yt_ps[:, j4 * 128:(j4 + 1) * 128], y_nat[:, j4, :], ident[:])
                        nc.vector.tensor_copy(
                            yT_buf[hl * 64:(hl + 1) * 64, b * T + jg * 512:b * T + (jg + 1) * 512],
                            yt_ps[:],
                        )

                # ---- batch-1 output projection at the very end ----
                if b == 1:
                    for ti in range(T // 128):
                        emit_outproj(T // 128 + ti)

    nc.compile()
    return nc


def _host_inputs(x, W_kqv, b_kqv, W_proj):
    """Build per-core input maps (host-side slicing/transposition)."""
    x = np.asarray(x, np.float32)
    W_kqv = np.asarray(W_kqv, np.float32)
    b_kqv = np.asarray(b_kqv, np.float32)
    W_proj = np.asarray(W_proj, np.float32)

    xT = np.ascontiguousarray(x.reshape(TT, C).T)  # (1024, 4096)
    sinT, cosT = _rope_tables()

    rmat = np.zeros((128, 128), np.float32)
    for hb in range(2):
        base = hb * 64
        for l in range(32):
            rmat[base + l, base + 32 + l] = -1.0
            rmat[base + 32 + l, base + l] = 1.0
    rperm = np.ascontiguousarray(rmat.T)  # lhsT

    # dmask[dcol]: (128, 512); cols [0, dcol*128) zero, diag block triu, rest ones
    dmask = np.zeros((128, 4 * 512), np.float32)
    triu = np.triu(np.ones((128, 128), np.float32))
    for dcol in range(4):
        base = dcol * 512
        dmask[:, base + dcol * 128: base + (dcol + 1) * 128] = triu
        dmask[:, base + (dcol + 1) * 128: base + 512] = 1.0
    ident = np.eye(128, dtype=np.float32)
    ident_dup = np.ascontiguousarray(
        np.concatenate([np.eye(64, dtype=np.float32), np.eye(64, dtype=np.float32)], axis=0))
    ones_col = np.zeros((128, 4 * KT_PER_B, 32), np.float32)
    ones_col[:, :, 0] = 1.0

    shared = {
        "xT": xT, "cosT": cosT, "sinT": sinT, "rperm": rperm,
        "dmask": dmask, "ident": ident, "ident_dup": ident_dup,
        "ones_col": ones_col,
    }

    in_maps = []
    for c in range(N_CORES):
        heads = [HPC * c + h for h in range(HPC)]

        def rows(mat, base):
            return np.concatenate([mat[base + h * D: base + (h + 1) * D] for h in heads], axis=0)

        Wk, Wq, Wv = rows(W_kqv, 0), rows(W_kqv, C), rows(W_kqv, 2 * C)
        w_sb = np.empty((128, NCT * 384), np.float32)
        for ct in range(NCT):
            sl = slice(ct * 128, (ct + 1) * 128)
            w_sb[:, ct * 384 + 0:ct * 384 + 128] = Wk[:, sl].T
            w_sb[:, ct * 384 + 128:ct * 384 + 256] = Wq[:, sl].T
            w_sb[:, ct * 384 + 256:ct * 384 + 384] = Wv[:, sl].T
        b_core = np.stack([
            np.concatenate([b_kqv[0 * C + h * D: 0 * C + (h + 1) * D] for h in heads]),
            np.concatenate([b_kqv[1 * C + h * D: 1 * C + (h + 1) * D] for h in heads]),
            np.concatenate([b_kqv[2 * C + h * D: 2 * C + (h + 1) * D] for h in heads]),
        ], axis=1).astype(np.float32)  # (128, 3)
        ch = np.concatenate([np.arange(h * D, (h + 1) * D) for h in heads])
        wprojT = np.ascontiguousarray(W_proj[:, ch].T)  # (128, 1024)

        in_maps.append({**shared, "w_sb": np.ascontiguousarray(w_sb),
                        "b_sb": b_core, "wprojT": wprojT})
    return in_maps


def kernel(x, padding_mask, W_kqv, b_kqv, W_proj, b_proj):
    padding_mask = np.asarray(padding_mask)
    if not padding_mask.all():
        return _numpy_fallback(x, padding_mask, W_kqv, b_kqv, W_proj, b_proj)

    if "nc" not in _CACHE:
        _CACHE["nc"] = build_program()
    nc = _CACHE["nc"]

    in_maps = _host_inputs(x, W_kqv, b_kqv, W_proj)
    res = run_bass_kernel_spmd(nc, in_maps, core_ids=list(range(N_CORES)))
    out = np.zeros((TT, C), np.float64)
    for c in range(N_CORES):
        out += res.results[c]["out_part"]
    out = (out + np.asarray(b_proj, np.float64)).astype(np.float32)
    return out.reshape(B, T, C)


if __name__ == "__main__":
    rng = np.random.default_rng(0)
    x = rng.standard_normal((B, T, C), dtype=np.float32)
    pm = np.ones((B, T), bool)
    W_kqv = (rng.standard_normal((3 * C, C), dtype=np.float32) / math.sqrt(C)).astype(np.float32)
    b_kqv = np.zeros((3 * C,), np.float32)
    W_proj = (rng.standard_normal((C, C), dtype=np.float32) / math.sqrt(C)).astype(np.float32)
    b_proj = np.zeros((C,), np.float32)
    out = kernel(x, pm, W_kqv, b_kqv, W_proj, b_proj)
    ref = _numpy_fallback(x, pm, W_kqv, b_kqv, W_proj, b_proj)
    err = np.abs(out - ref).max() / np.abs(ref).max()
    print("self-check rel err:", err)


# revision 20
# speedup vs baseline: 1.2140x; 1.0132x over previous
"""Trainium2 Bass kernel for causal self-attention with RoPE.

Contract: kernel(**inputs) takes the FULL unsharded inputs of
nn_CausalSelfAttention (x (2,2048,1024) f32, padding_mask (2,2048) bool,
W_kqv (3072,1024), b_kqv (3072,), W_proj (1024,1024), b_proj (1024,))
and returns the full (2,2048,1024) f32 output.

Sharding: 16 heads x 2 batches = 32 (batch, head) pairs, 4 per core
(2 heads, both batches). Each core computes the QKV projection for its
2 heads only (weights pre-sliced host-side), attention for its 4 pairs,
and a partial output projection over its 128 channels of the 1024-wide
contraction. The host sums the 8 partial outputs (no collectives).
"""

import sys

for _p in ("/opt/trn_rl_repo",):
    if _p not in sys.path:
        sys.path.append(_p)

import math

import numpy as np

import concourse.bass as bass
import concourse.tile as tile
from concourse import bacc, mybir
from concourse.bass_utils import run_bass_kernel_spmd

# Problem constants (hardcoded per spec).
B, T, C = 2, 2048, 1024
H, D = 16, 64
N_CORES = 8
HPC = H // N_CORES          # heads per core = 2
TT = B * T                  # 4096
NCT = C // 128              # 8 c-tiles
CHUNK = 512                 # t-chunk width (phase A) and q-chunk width (phase B)
NCHUNK = TT // CHUNK        # 8
KT_PER_B = T // 128         # 16 k-tiles per batch
SCALE = 1.0 / math.sqrt(D)
ROPE_BASE = 10000.0

F32 = mybir.dt.float32
MMDT = mybir.dt.float32r    # reduced-precision fp32 matmul mode (4x faster)

_CACHE = {}


def _rope_tables():
    half = D // 2
    inv_freq = (np.float32(ROPE_BASE) ** (-(np.arange(half, dtype=np.float32) / np.float32(half)))).astype(np.float32)
    t = np.arange(T, dtype=np.float32)[:, None]
    ang = t * inv_freq[None, :]                       # (T, 32)
    sin = np.concatenate([np.sin(ang), np.sin(ang)], axis=-1)  # (T, 64)
    cos = np.concatenate([np.cos(ang), np.cos(ang)], axis=-1)
    # transpose to (64, T), duplicate along partitions for the 2 heads of an M-tile
    sinT = np.ascontiguousarray(np.concatenate([sin.T, sin.T], axis=0))  # (128, T)
    cosT = np.ascontiguousarray(np.concatenate([cos.T, cos.T], axis=0))
    return sinT, cosT


def _numpy_fallback(x, padding_mask, W_kqv, b_kqv, W_proj, b_proj):
    """Exact reference in numpy — used only for non-all-ones padding masks."""
    x = np.asarray(x, np.float32)
    qkv = x.reshape(TT, C) @ np.asarray(W_kqv, np.float32).T + np.asarray(b_kqv, np.float32)
    qkv = qkv.reshape(B, T, 3 * C)
    k, q, v = np.split(qkv, 3, axis=-1)

    def split_heads(t):
        return t.reshape(B, T, H, D).transpose(0, 2, 1, 3)

    q, k, v = split_heads(q), split_heads(k), split_heads(v)
    sinT, cosT = _rope_tables()
    sin = sinT[:D].T[None, None]
    cos = cosT[:D].T[None, None]

    def rot(t):
        return np.concatenate([-t[..., D // 2:], t[..., : D // 2]], axis=-1)

    q = q * cos + rot(q) * sin
    k = k * cos + rot(k) * sin
    att = np.einsum("bhqd,bhkd->bhqk", q, k) * SCALE
    causal = np.tril(np.ones((T, T), bool))[None, None]
    mask = causal & np.asarray(padding_mask)[:, None, None, :]
    att = np.where(mask, att, -np.inf)
    att = att - att.max(axis=-1, keepdims=True)
    e = np.exp(att)
    p = e / e.sum(axis=-1, keepdims=True)
    y = np.einsum("bhqk,bhkd->bhqd", p, v)
    y = y * np.asarray(padding_mask)[:, None, :, None]
    y = y.transpose(0, 2, 1, 3).reshape(B, T, C)
    return (y @ np.asarray(W_proj, np.float32).T + np.asarray(b_proj, np.float32)).astype(np.float32)


def build_program():
    nc = bacc.Bacc("TRN2", target_bir_lowering=False, debug=False, num_devices=N_CORES)

    # ---- I/O ----
    xT = nc.dram_tensor("xT", [C, TT], MMDT, kind="ExternalInput").ap()
    w_sb_d = nc.dram_tensor("w_sb", [128, NCT * 384], MMDT, kind="ExternalInput").ap()
    b_sb_d = nc.dram_tensor("b_sb", [128, 3], F32, kind="ExternalInput").ap()
    wproj_d = nc.dram_tensor("wprojT", [128, C], MMDT, kind="ExternalInput").ap()
    cos_d = nc.dram_tensor("cosT", [128, T], MMDT, kind="ExternalInput").ap()
    sin_d = nc.dram_tensor("sinT", [128, T], MMDT, kind="ExternalInput").ap()
    rperm_d = nc.dram_tensor("rperm", [128, 128], MMDT, kind="ExternalInput").ap()
    dmask_d = nc.dram_tensor("dmask", [128, 4 * 512], MMDT, kind="ExternalInput").ap()
    ident_d = nc.dram_tensor("ident", [128, 128], MMDT, kind="ExternalInput").ap()
    identd_d = nc.dram_tensor("ident_dup", [128, 64], MMDT, kind="ExternalInput").ap()
    ones_d = nc.dram_tensor("ones_col", [128, 4 * KT_PER_B, 32], MMDT, kind="ExternalInput").ap()
    out_d = nc.dram_tensor("out_part", [TT, C], F32, kind="ExternalOutput").ap()

    Exp = mybir.ActivationFunctionType.Exp
    Copy = mybir.ActivationFunctionType.Copy

    with tile.TileContext(nc) as tc:
        with (
            tc.tile_pool(name="const", bufs=1) as cpool,
            tc.tile_pool(name="persist", bufs=1) as ppool,
            tc.tile_pool(name="outT", bufs=2) as opool,
            tc.tile_pool(name="work", bufs=3) as wpool,
            tc.tile_pool(name="ptp", bufs=4) as ptp,
            tc.tile_pool(name="xtp", bufs=12) as xtp,
            tc.tile_pool(name="work2", bufs=2) as w2pool,
            tc.tile_pool(name="psum", bufs=2, space="PSUM") as ps,
            tc.tile_pool(name="psum_av", bufs=1, space="PSUM") as ps_av,
        ):
            # ---- weights first (matmuls start as soon as block 0 lands) ----
            w_t = []
            for ct in range(NCT):
                wt = cpool.tile([128, 384], MMDT, tag=f"w{ct}")
                nc.sync.dma_start(wt[:], w_sb_d[:, ct * 384:(ct + 1) * 384])
                w_t.append(wt)
            b_sb = cpool.tile([128, 3], F32, tag="b")
            nc.sync.dma_start(b_sb[:], b_sb_d[:])
            cosT = cpool.tile([128, T], MMDT, tag="cos")
            nc.sync.dma_start(cosT[:], cos_d[:])
            sinT = cpool.tile([128, T], MMDT, tag="sin")
            nc.sync.dma_start(sinT[:], sin_d[:])
            rperm = cpool.tile([128, 128], MMDT, tag="rp")
            nc.sync.dma_start(rperm[:], rperm_d[:])
            identd = cpool.tile([128, 64], MMDT, tag="idd")
            nc.sync.dma_start(identd[:], identd_d[:])
            wproj = cpool.tile([128, C], MMDT, tag="wp")
            nc.sync.dma_start(wproj[:], wproj_d[:])
            dmask = cpool.tile([128, 4 * 512], MMDT, tag="dm")
            nc.sync.dma_start(dmask[:], dmask_d[:])
            ident = cpool.tile([128, 128], MMDT, tag="id")
            nc.sync.dma_start(ident[:], ident_d[:])

            # ---- persistent buffers ----
            kT_buf = ppool.tile([128, TT], MMDT, tag="kT")
            qT_buf = ppool.tile([128, TT], MMDT, tag="qT")
            v_ext = ppool.tile([128, 4 * KT_PER_B, 96], MMDT, tag="vx")
            yT_buf = ppool.tile([128, TT], MMDT, tag="yT")

            # ones column for the softmax-denominator trick
            nc.sync.dma_start(v_ext[:, :, 64:96], ones_d[:])

            # ================= Phase A: projection + RoPE + V layout ==========
            def emit_proj_chunk(i):
                b = i // (NCHUNK // B)
                tb = (i % (NCHUNK // B)) * CHUNK  # within-batch t offset
                ps_kq = ps.tile([128, 1024], F32, tag="big")
                ps_v = ps.tile([128, 512], F32, tag="tr")
                for ct in range(NCT):
                    xt = xtp.tile([128, CHUNK], MMDT, tag="xt")
                    nc.sync.dma_start(xt[:], xT[ct * 128:(ct + 1) * 128, i * CHUNK:(i + 1) * CHUNK])
                    st, sp = (ct == 0), (ct == NCT - 1)
                    nc.tensor.matmul(ps_kq[:, 0:512], w_t[ct][:, 0:128], xt[:], start=st, stop=sp)
                    nc.tensor.matmul(ps_kq[:, 512:1024], w_t[ct][:, 128:256], xt[:], start=st, stop=sp)
                    nc.tensor.matmul(ps_v[:], w_t[ct][:, 256:384], xt[:], start=st, stop=sp)

                k_raw = w2pool.tile([128, CHUNK], MMDT, tag="kraw")
                q_raw = w2pool.tile([128, CHUNK], MMDT, tag="qraw")
                v_raw = w2pool.tile([128, CHUNK], MMDT, tag="vraw")
                nc.vector.tensor_scalar_add(k_raw[:], ps_kq[:, 0:512], b_sb[:, 0:1])
                nc.vector.tensor_scalar_add(q_raw[:], ps_kq[:, 512:1024], b_sb[:, 1:2])
                nc.vector.tensor_scalar_add(v_raw[:], ps_v[:], b_sb[:, 2:3])

                # rotate_half via +-1 permutation matmul, then q' = q*cos + rot*sin
                ps_rot = ps_av.tile([128, 1024], F32, tag="av")
                ps_krot = ps_rot[:, 0:512]
                ps_qrot = ps_rot[:, 512:1024]
                nc.tensor.matmul(ps_krot[:], rperm[:], k_raw[:], start=True, stop=True)
                nc.tensor.matmul(ps_qrot[:], rperm[:], q_raw[:], start=True, stop=True)
                cs = cosT[:, tb:tb + CHUNK]
                sn = sinT[:, tb:tb + CHUNK]
                tmp1 = w2pool.tile([128, CHUNK], MMDT, tag="tmp1")
                tmp2 = w2pool.tile([128, CHUNK], MMDT, tag="tmp2")
                nc.vector.tensor_mul(tmp1[:], k_raw[:], cs)
                nc.vector.tensor_mul(tmp2[:], ps_krot[:], sn)
                nc.vector.tensor_add(kT_buf[:, i * CHUNK:(i + 1) * CHUNK], tmp1[:], tmp2[:])
                tmp3 = w2pool.tile([128, CHUNK], MMDT, tag="tmp1")
                tmp4 = w2pool.tile([128, CHUNK], MMDT, tag="tmp2")
                nc.vector.tensor_mul(tmp3[:], q_raw[:], cs)
                nc.vector.tensor_mul(tmp4[:], ps_qrot[:], sn)
                nc.vector.tensor_add(qT_buf[:, i * CHUNK:(i + 1) * CHUNK], tmp3[:], tmp4[:])

                # V: transpose (64,128) slabs into v_ext natural layout (batched copy)
                for hl in range(HPC):
                    p = b * HPC + hl
                    kt0 = (i % (NCHUNK // B)) * (CHUNK // 128)  # first k-tile of chunk
                    vtr = ps.tile([128, 4, 64], MMDT, tag="tr")
                    for j in range(CHUNK // 128):
                        nc.tensor.transpose(
                            vtr[:, j, :], v_raw[hl * 64:(hl + 1) * 64, j * 128:(j + 1) * 128],
                            identd[hl * 64:(hl + 1) * 64, :],
                        )
                    nc.vector.tensor_copy(
                        v_ext[:, p * KT_PER_B + kt0:p * KT_PER_B + kt0 + 4, 0:64], vtr[:])

            # ================= Phase B/C helpers ==============================
            def emit_outproj(tg):
                out_sb = wpool.tile([128, C], F32, tag="osb")
                for half in range(2):
                    op_ps = ps.tile([128, 512], F32, tag="tr")
                    nc.tensor.matmul(
                        op_ps[:],
                        yT_buf[:, tg * 128:(tg + 1) * 128],
                        wproj[:, half * 512:(half + 1) * 512],
                        start=True, stop=True,
                    )
                    if half == 0:
                        nc.vector.tensor_copy(out_sb[:, 0:512], op_ps[:])
                    else:
                        nc.scalar.activation(out_sb[:, 512:1024], op_ps[:], Copy)
                nc.sync.dma_start(out_d[tg * 128:(tg + 1) * 128, :], out_sb[:])

            def emit_attention(b, hl, interleave_tiles, tail_tiles):
                """Attention for pair (b, hl). interleave_tiles: out-proj t-tiles
                to emit spread across the qc loop; tail_tiles: t-tile groups to
                emit inside the pair tail (4 per jg iteration)."""
                p = b * HPC + hl
                QT = qT_buf[hl * 64:(hl + 1) * 64, b * T:(b + 1) * T]
                KT = kT_buf[hl * 64:(hl + 1) * 64, b * T:(b + 1) * T]
                outT = opool.tile([96, T], MMDT, tag="outT")
                n_qc = T // CHUNK
                per_qc = len(interleave_tiles) // n_qc if interleave_tiles else 0
                for qc in range(n_qc):
                    n_kt = 4 * (qc + 1)
                    # double-banked accumulator: even k-tiles -> bank A, odd -> B
                    av_ps = ps_av.tile([96, 1024], F32, tag="av")
                    pending = []  # [(pt, kjs), ...]

                    def flush_av(drain):
                        while pending and (len(pending) > 2 or drain):
                            ppt, pkjs = pending.pop(0)
                            for idx, kj in enumerate(pkjs):
                                bank = (kj % 2) * 512
                                nc.tensor.matmul(
                                    av_ps[:, bank:bank + 512],
                                    v_ext[:, p * KT_PER_B + kj, :],
                                    ppt[:, idx * 512:(idx + 1) * 512],
                                    start=(kj < 2), stop=(kj >= n_kt - 2),
                                    skip_group_check=True,
                                )

                    for g0 in range(0, n_kt, 2):
                        kjs = list(range(g0, min(g0 + 2, n_kt)))
                        gw = len(kjs) * 512
                        s_ps = ps.tile([128, 1024], F32, tag="big")
                        for idx, kj in enumerate(kjs):
                            nc.tensor.matmul(
                                s_ps[:, idx * 512:(idx + 1) * 512],
                                KT[:, kj * 128:(kj + 1) * 128],
                                QT[:, qc * CHUNK:(qc + 1) * CHUNK],
                                start=True, stop=True,
                            )
                        pt = ptp.tile([128, 1024], MMDT, tag="pt")
                        nc.scalar.activation(pt[:, 0:gw], s_ps[:, 0:gw], Exp, scale=SCALE)
                        for idx, kj in enumerate(kjs):
                            if kj >= 4 * qc:  # diagonal-region k-tile
                                dcol = kj - 4 * qc
                                off = idx * 512
                                w = (dcol + 1) * 128
                                nc.gpsimd.tensor_mul(
                                    pt[:, off:off + w],
                                    pt[:, off:off + w],
                                    dmask[:, dcol * 512:dcol * 512 + w],
                                )
                        pending.append((pt, kjs))
                        flush_av(False)
                    flush_av(True)
                    oslice = outT[:, qc * CHUNK:(qc + 1) * CHUNK]
                    nc.vector.tensor_copy(oslice, av_ps[:, 0:512])
                    nc.vector.tensor_add(oslice, oslice, av_ps[:, 512:1024])
                    for tg in interleave_tiles[qc * per_qc:(qc + 1) * per_qc]:
                        emit_outproj(tg)

                # pair tail: transpose, normalize, transpose back into yT_buf
                for jg in range(T // 512):
                    nat = ps.tile([128, 4, 96], MMDT, tag="tr")
                    for j4 in range(4):
                        nc.tensor.transpose(
                            nat[:, j4, :], outT[:, (jg * 4 + j4) * 128:(jg * 4 + j4 + 1) * 128],
                            ident[0:96, 0:96])
                    recip = wpool.tile([128, 4, 1], F32, tag="rcp")
                    nc.vector.reciprocal(recip[:], nat[:, :, 64:65])
                    y_nat = wpool.tile([128, 4, 64], MMDT, tag="ynat")
                    for j4 in range(4):
                        nc.vector.tensor_scalar_mul(
                            y_nat[:, j4, :], nat[:, j4, 0:64], recip[:, j4, :])
                    yt_ps = ps.tile([64, 512], MMDT, tag="tr")
                    for j4 in range(4):
                        nc.tensor.transpose(yt_ps[:, j4 * 128:(j4 + 1) * 128], y_nat[:, j4, :], ident[:])
                    nc.vector.tensor_copy(
                        yT_buf[hl * 64:(hl + 1) * 64, b * T + jg * 512:b * T + (jg + 1) * 512],
                        yt_ps[:],
                    )
                    if tail_tiles:
                        for tg in tail_tiles[jg * 4:(jg + 1) * 4]:
                            emit_outproj(tg)

            # ============== orchestration: overlap proj DMA with attention ====
            NPB = NCHUNK // B  # proj chunks per batch
            NT = T // 128      # out-proj t-tiles per batch
            for i in range(NPB):
                emit_proj_chunk(i)                       # batch 0 projection
            emit_attention(0, 0, [], [])
            for i in range(NPB, 2 * NPB):
                emit_proj_chunk(i)                       # batch 1 projection (DMA overlaps)
            emit_attention(0, 1, [], [])
            emit_attention(1, 0, list(range(0, NT // 2)), [])
            emit_attention(1, 1, list(range(NT // 2, NT)), list(range(NT, 2 * NT)))

    nc.compile()
    return nc


def _host_inputs(x, W_kqv, b_kqv, W_proj):
    """Build per-core input maps (host-side slicing/transposition)."""
    x = np.asarray(x, np.float32)
    W_kqv = np.asarray(W_kqv, np.float32)
    b_kqv = np.asarray(b_kqv, np.float32)
    W_proj = np.asarray(W_proj, np.float32)

    xT = np.ascontiguousarray(x.reshape(TT, C).T)  # (1024, 4096)
    sinT, cosT = _rope_tables()

    rmat = np.zeros((128, 128), np.float32)
    for hb in range(2):
        base = hb * 64
        for l in range(32):
            rmat[base + l, base + 32 + l] = -1.0
            rmat[base + 32 + l, base + l] = 1.0
    rperm = np.ascontiguousarray(rmat.T)  # lhsT

    # dmask[dcol]: (128, 512); cols [0, dcol*128) zero, diag block triu, rest ones
    dmask = np.zeros((128, 4 * 512), np.float32)
    triu = np.triu(np.ones((128, 128), np.float32))
    for dcol in range(4):
        base = dcol * 512
        dmask[:, base + dcol * 128: base + (dcol + 1) * 128] = triu
        dmask[:, base + (dcol + 1) * 128: base + 512] = 1.0
    ident = np.eye(128, dtype=np.float32)
    ident_dup = np.ascontiguousarray(
        np.concatenate([np.eye(64, dtype=np.float32), np.eye(64, dtype=np.float32)], axis=0))
    ones_col = np.zeros((128, 4 * KT_PER_B, 32), np.float32)
    ones_col[:, :, 0] = 1.0

    shared = {
        "xT": xT, "cosT": cosT, "sinT": sinT, "rperm": rperm,
        "dmask": dmask, "ident": ident, "ident_dup": ident_dup,
        "ones_col": ones_col,
    }

    in_maps = []
    for c in range(N_CORES):
        heads = [HPC * c + h for h in range(HPC)]

        def rows(mat, base):
            return np.concatenate([mat[base + h * D: base + (h + 1) * D] for h in heads], axis=0)

        Wk, Wq, Wv = rows(W_kqv, 0), rows(W_kqv, C), rows(W_kqv, 2 * C)
        w_sb = np.empty((128, NCT * 384), np.float32)
        for ct in range(NCT):
            sl = slice(ct * 128, (ct + 1) * 128)
            w_sb[:, ct * 384 + 0:ct * 384 + 128] = Wk[:, sl].T
            w_sb[:, ct * 384 + 128:ct * 384 + 256] = Wq[:, sl].T
            w_sb[:, ct * 384 + 256:ct * 384 + 384] = Wv[:, sl].T
        b_core = np.stack([
            np.concatenate([b_kqv[0 * C + h * D: 0 * C + (h + 1) * D] for h in heads]),
            np.concatenate([b_kqv[1 * C + h * D: 1 * C + (h + 1) * D] for h in heads]),
            np.concatenate([b_kqv[2 * C + h * D: 2 * C + (h + 1) * D] for h in heads]),
        ], axis=1).astype(np.float32)  # (128, 3)
        ch = np.concatenate([np.arange(h * D, (h + 1) * D) for h in heads])
        wprojT = np.ascontiguousarray(W_proj[:, ch].T)  # (128, 1024)

        in_maps.append({**shared, "w_sb": np.ascontiguousarray(w_sb),
                        "b_sb": b_core, "wprojT": wprojT})
    return in_maps


def kernel(x, padding_mask, W_kqv, b_kqv, W_proj, b_proj):
    padding_mask = np.asarray(padding_mask)
    if not padding_mask.all():
        return _numpy_fallback(x, padding_mask, W_kqv, b_kqv, W_proj, b_proj)

    if "nc" not in _CACHE:
        _CACHE["nc"] = build_program()
    nc = _CACHE["nc"]

    in_maps = _host_inputs(x, W_kqv, b_kqv, W_proj)
    res = run_bass_kernel_spmd(nc, in_maps, core_ids=list(range(N_CORES)))
    out = np.zeros((TT, C), np.float64)
    for c in range(N_CORES):
        out += res.results[c]["out_part"]
    out = (out + np.asarray(b_proj, np.float64)).astype(np.float32)
    return out.reshape(B, T, C)


if __name__ == "__main__":
    rng = np.random.default_rng(0)
    x = rng.standard_normal((B, T, C), dtype=np.float32)
    pm = np.ones((B, T), bool)
    W_kqv = (rng.standard_normal((3 * C, C), dtype=np.float32) / math.sqrt(C)).astype(np.float32)
    b_kqv = np.zeros((3 * C,), np.float32)
    W_proj = (rng.standard_normal((C, C), dtype=np.float32) / math.sqrt(C)).astype(np.float32)
    b_proj = np.zeros((C,), np.float32)
    out = kernel(x, pm, W_kqv, b_kqv, W_proj, b_proj)
    ref = _numpy_fallback(x, pm, W_kqv, b_kqv, W_proj, b_proj)
    err = np.abs(out - ref).max() / np.abs(ref).max()
    print("self-check rel err:", err)
